# revision 1
# baseline (speedup 1.0000x reference)
"""LocallyConnected2d v4: all-DVE — fused fp16 products + tree accumulation.

HW measurements showed per-instruction overheads dominate (DVE op ~0.3-0.5us
fixed, PE matmul ~1.1us fixed), so v4 minimizes instruction count and uses no
TensorEngine/PSUM at all:

- Products: kh decomposes as 4*q2 + rp (+2 for phase 1), so one tensor_tensor
  with free dims (kw, q2, b, i) covers 4 kw-columns x 2 kh taps at once; the
  weight operand broadcasts over b with a stride-0 dim; innermost dim is i
  (stride 1, even offsets) so the DVE fp16 2x_1p mode stays engaged.
- Tap accumulation is a fused fp16 add-tree on the DVE.
- kw runs in two halves (4+3 columns) so all arenas fit SBUF with unique
  bufs=1 tags (no pool-slot recycling -> no scheduler deadlocks).
~26 DVE ops total per iteration.
"""

import sys

sys.path.insert(0, "/opt/trn_rl_repo")

import numpy as np

import concourse.bass as bass
import concourse.bacc as bacc
import concourse.mybir as mybir
from concourse.tile import TileContext
from concourse.bass_utils import run_bass_kernel_spmd

B = 64
H = W = 224
KH = KW = 7
PH = PW = 3
NKH = NKW = 112
NCORES = 8
RPC = NKH // NCORES       # 14
R2 = 16
SLAB_ROWS = 2 * (RPC - 1) + KH  # 33

F16 = mybir.dt.float16
F32 = mybir.dt.float32


def _ap(base, dims):
    return bass.AP(tensor=base.tensor, offset=base.offset,
                   ap=[base.ap[0]] + dims)


def _build_nc(n_iters=1):
    nc = bacc.Bacc("TRN2", target_bir_lowering=False, debug=False,
                   num_devices=NCORES)

    xp_d = [nc.dram_tensor(f"xp{p}", [NKW, KW, 2, B, R2], F16,
                           kind="ExternalInput") for p in range(2)]
    w_d = nc.dram_tensor("w", [NKW, KH, KW, RPC], F16, kind="ExternalInput")
    b_d = nc.dram_tensor("bias", [NKW, RPC], F16, kind="ExternalInput")
    o_d = nc.dram_tensor("o", [NKW, B * RPC], F32, kind="ExternalOutput")

    KWR = KW * RPC              # kh stride in w (elements)
    SKW, SB = 2 * B * R2, R2    # x store strides (elements)
    HALVES = [(0, 4), (4, 3)]   # kw ranges

    with TileContext(nc) as tc:
        with tc.tile_pool(name="pool", bufs=1) as pool:
            xs = [pool.tile([NKW, KW, 2, B, R2], F16, tag=f"x{p}",
                            name=f"xs{p}") for p in range(2)]
            wt = pool.tile([NKW, KH, KW, RPC], F16, tag="w")
            bt = pool.tile([NKW, RPC], F16, tag="b")
            osb = pool.tile([NKW, B * RPC], F32, tag="osb")
            # arenas, unique tags, sized for the bigger (4-kw) half
            A = [pool.tile([NKW, 4, 2, B, RPC], F16, tag=f"A{i}",
                           name=f"A{i}") for i in range(3)]
            Sm = [pool.tile([NKW, 4, B, RPC], F16, tag=f"Sm{i}",
                            name=f"Sm{i}") for i in range(7)]
            Sh = [pool.tile([NKW, 4, B, RPC], F16, tag=f"Sh{i}",
                            name=f"Sh{i}") for i in range(2)]
            u = [pool.tile([NKW, 2, B, RPC], F16, tag=f"u{i}",
                           name=f"u{i}") for i in range(4)]

            for it in range(n_iters):
                nc.gpsimd.dma_start(out=wt[:, :, :, :], in_=w_d.ap())
                nc.gpsimd.dma_start(out=bt[:, :], in_=b_d.ap())
                for p in range(2):
                    for c0, cn in HALVES:
                        nc.gpsimd.dma_start(
                            out=xs[p][:, c0:c0 + cn, :, :, :],
                            in_=xp_d[p].ap()[:, c0:c0 + cn])

                for h, (c0, cn) in enumerate(HALVES):
                    # products: (x-store phase, rp, base kh) per arena
                    specs = [(0, 0, 0), (0, 1, 1), (1, 0, 2)]
                    for ai, (p, rp, kh0) in enumerate(specs):
                        for q2 in range(2):
                            xb = xs[p][:, c0, rp, :, 2 * q2:2 * q2 + 1]
                            wb = wt[:, kh0 + 4 * q2, c0, 0:1]
                            nc.vector.tensor_tensor(
                                out=A[ai][:, 0:cn, q2, :, :],
                                in0=_ap(xb, [[SKW, cn], [SB, B], [1, RPC]]),
                                in1=_ap(wb, [[RPC, cn], [0, B], [1, RPC]]),
                                op=mybir.AluOpType.mult)
                    # A2 (kh=3) straight into Sm[3]
                    xb = xs[1][:, c0, 1, :, 0:1]
                    wb = wt[:, 3, c0, 0:1]
                    nc.vector.tensor_tensor(
                        out=Sm[3][:, 0:cn, :, :],
                        in0=_ap(xb, [[SKW, cn], [SB, B], [1, RPC]]),
                        in1=_ap(wb, [[RPC, cn], [0, B], [1, RPC]]),
                        op=mybir.AluOpType.mult)
                    # q2 sums
                    for ai in range(3):
                        nc.vector.tensor_add(out=Sm[ai][:, 0:cn, :, :],
                                             in0=A[ai][:, 0:cn, 0, :, :],
                                             in1=A[ai][:, 0:cn, 1, :, :])
                    # merge 4 -> 1
                    nc.vector.tensor_add(out=Sm[4][:, 0:cn, :, :],
                                         in0=Sm[0][:, 0:cn, :, :],
                                         in1=Sm[1][:, 0:cn, :, :])
                    nc.vector.tensor_add(out=Sm[5][:, 0:cn, :, :],
                                         in0=Sm[2][:, 0:cn, :, :],
                                         in1=Sm[3][:, 0:cn, :, :])
                    nc.vector.tensor_add(out=Sh[h][:, 0:cn, :, :],
                                         in0=Sm[4][:, 0:cn, :, :],
                                         in1=Sm[5][:, 0:cn, :, :])

                # kw tree: Sh[0] has 4 partials, Sh[1] has 3
                nc.vector.tensor_add(out=u[0][:, :, :, :],
                                     in0=Sh[0][:, 0:2, :, :],
                                     in1=Sh[0][:, 2:4, :, :])
                nc.vector.tensor_add(out=u[1][:, 0, :, :],
                                     in0=u[0][:, 0, :, :],
                                     in1=u[0][:, 1, :, :])
                nc.vector.tensor_add(out=u[2][:, 0, :, :],
                                     in0=Sh[1][:, 0, :, :],
                                     in1=Sh[1][:, 1, :, :])
                nc.vector.tensor_add(out=u[3][:, 0, :, :],
                                     in0=u[1][:, 0, :, :],
                                     in1=u[2][:, 0, :, :])
                nc.vector.tensor_add(out=u[1][:, 1, :, :],
                                     in0=u[3][:, 0, :, :],
                                     in1=Sh[1][:, 2, :, :])

                # epilogue: f32 out = partial + bias (broadcast over b)
                bias_b = bt[:, :].unsqueeze(1).broadcast_to([NKW, B, RPC])
                out_v = osb[:, :].rearrange("p (b i) -> p b i", i=RPC)
                nc.vector.tensor_add(out=out_v, in0=u[1][:, 1, :, :],
                                     in1=bias_b)

                nc.gpsimd.dma_start(out=o_d.ap(), in_=osb[:, :])

    nc.compile()
    return nc


def _shard_inputs(x, weights, bias):
    x = np.asarray(x, dtype=np.float32)
    weights = np.asarray(weights, dtype=np.float32)
    bias = np.asarray(bias, dtype=np.float32)

    x_pad = np.zeros((B, H + 2 * PH, W + 2 * PW), dtype=np.float32)
    x_pad[:, PH:PH + H, PW:PW + W] = x
    x_pad = x_pad.astype(np.float16)

    in_maps = []
    for c in range(NCORES):
        r0 = 2 * RPC * c
        slab = x_pad[:, r0:r0 + SLAB_ROWS, :]
        slab_p = np.zeros((B, 2 * (R2 + 1) + 1, 230), dtype=np.float16)
        slab_p[:, :SLAB_ROWS] = slab
        m = {}
        for p in range(2):
            xp = np.empty((NKW, KW, 2, B, R2), dtype=np.float16)
            for k in range(KW):
                for rp in range(2):
                    rows = slab_p[:, 2 * p + rp:2 * p + rp + 2 * R2:2,
                                  k:k + 2 * NKW:2]
                    xp[:, k, rp] = rows.transpose(2, 0, 1)
            m[f"xp{p}"] = xp
        m["w"] = np.ascontiguousarray(
            weights[RPC * c:RPC * (c + 1)].transpose(1, 2, 3, 0)
        ).astype(np.float16)
        m["bias"] = np.ascontiguousarray(
            bias[RPC * c:RPC * (c + 1)].T).astype(np.float16)
        in_maps.append(m)
    return in_maps


def _unshard_output(results):
    o_all = np.stack([r["o"].reshape(NKW, B, RPC) for r in results])
    return np.ascontiguousarray(o_all.transpose(2, 0, 3, 1)).reshape(B, NKH, NKW)


def make_runner(nc, in_maps):
    """Build a cached jitted runner for nc; returns (run, out_names).
    run() re-executes the NEFF without rebuilding the jit wrapper, so
    repeated calls measure dispatch+exec only."""
    import jax
    import jax.numpy as jnp
    from jax.sharding import Mesh, PartitionSpec
    from jax.experimental.shard_map import shard_map
    import concourse.mybir as mybir
    from concourse.bass2jax import (_bass_exec_p, install_neuronx_cc_hook,
                                    partition_id_tensor)

    install_neuronx_cc_hook()
    n_cores = len(in_maps)
    partition_name = (nc.partition_id_tensor.name
                      if nc.partition_id_tensor else None)
    in_names, out_names, out_avals, zero_outs = [], [], [], []
    for alloc in nc.m.functions[0].allocations:
        if not isinstance(alloc, mybir.MemoryLocationSet):
            continue
        name = alloc.memorylocations[0].name
        if alloc.kind == "ExternalInput":
            if name != partition_name:
                in_names.append(name)
        elif alloc.kind == "ExternalOutput":
            shape = tuple(alloc.tensor_shape)
            dtype = mybir.dt.np(alloc.dtype)
            out_names.append(name)
            out_avals.append(jax.core.ShapedArray(shape, dtype))
            zero_outs.append(np.zeros(shape, dtype))
    n_params = len(in_names)
    all_in_names = list(in_names) + list(out_names)
    if partition_name is not None:
        all_in_names.append(partition_name)

    def _body(*args):
        operands = list(args)
        if partition_name is not None:
            operands.append(partition_id_tensor())
        return tuple(_bass_exec_p.bind(
            *operands, out_avals=tuple(out_avals),
            in_names=tuple(all_in_names), out_names=tuple(out_names),
            lowering_input_output_aliases=(), sim_require_finite=True,
            sim_require_nnan=True, nc=nc))

    devices = jax.devices()[:n_cores]
    mesh = Mesh(np.asarray(devices), ("core",))
    n_outs = len(out_names)
    sharded = jax.jit(
        shard_map(_body, mesh=mesh,
                  in_specs=(PartitionSpec("core"),) * (n_params + n_outs),
                  out_specs=(PartitionSpec("core"),) * n_outs,
                  check_rep=False),
        donate_argnums=tuple(range(n_params, n_params + n_outs)),
        keep_unused=True)

    concat_in = [np.concatenate([np.asarray(in_maps[c][nm])
                                 for c in range(n_cores)], axis=0)
                 for nm in in_names]
    concat_in = [jax.device_put(a) for a in concat_in]

    def run():
        zeros = [np.zeros((n_cores * z.shape[0], *z.shape[1:]), z.dtype)
                 for z in zero_outs]
        outs = sharded(*concat_in, *zeros)
        jax.block_until_ready(outs)
        return outs

    def unpack(outs):
        return [{nm: np.asarray(outs[i]).reshape(n_cores, *out_avals[i].shape)[c]
                 for i, nm in enumerate(out_names)} for c in range(n_cores)]

    return run, unpack


_NC_CACHE = None


def _get_nc():
    global _NC_CACHE
    if _NC_CACHE is None:
        _NC_CACHE = _build_nc()
    return _NC_CACHE


def kernel(x, weights, bias):
    nc = _get_nc()
    in_maps = _shard_inputs(x, weights, bias)
    res = run_bass_kernel_spmd(nc, in_maps, core_ids=list(range(NCORES)))
    return _unshard_output(res.results)


def benchmark(x, weights, bias, n_big=384, reps=15):
    import time

    in_maps = _shard_inputs(x, weights, bias)
    nc1 = _build_nc(1)
    run1, _ = make_runner(nc1, in_maps)
    ncN = _build_nc(n_big)
    runN, unpackN = make_runner(ncN, in_maps)
    run1(); outsN = runN()
    t1, tN = [], []
    for _ in range(reps):
        t0 = time.perf_counter(); run1(); t1.append(time.perf_counter() - t0)
        t0 = time.perf_counter(); runN(); tN.append(time.perf_counter() - t0)
    times = {1: min(t1), n_big: min(tN)}
    per_iter_ns = (times[n_big] - times[1]) / (n_big - 1) * 1e9
    return per_iter_ns, times, _unshard_output(unpackN(outsN))

